# revision 11
# baseline (speedup 1.0000x reference)
"""Trainium2 Bass kernel for sliding-window multi-head attention (v2).

Problem (nn_MultiHeadAttention_74285754352148):
  B=2, S=2048, D=1024, H=16, HD=64, WINDOW=512 (causal, j in [i-256, i]),
  RoPE theta=10000, out = softmax(mask(QK^T)/8) V @ Wo + bo.

Sharding: batch x sequence across 8 cores (core c: batch c//4, tokens
[512*(c%4), 512*(c%4)+512)). Each core recomputes K/V for a 256-token halo;
no collectives.

v2 structure (per core):
  - Projections optionally in fp8(e4m3) with DoubleRow perf mode (2 fp8
    weights/cell -> contraction 256/matmul), weights prescaled x32 on host.
  - RoPE: dim-major, psum-pair-fused DVE ops (u=ps*cos, w=ps*sin2), 32-shift
    via PE permutation matmul, add -> bf16 qrope/krope.
  - Attention in 128-query blocks x 3 key chunks of 128 (67% band
    utilization): scores via 2-head co-run (PE row groups 0-63/64-127),
    one exp per (head-pair, qblock), 2 gpsimd affine band masks covering
    both heads at once, ctx matmul with ones-column denominator.
  - Softmax normalization batched 4 heads/psum bank: one subtract + one
    reciprocal per group, scaled psum->sbuf copies alternating ACT/DVE.
  - ctx -> ctxT via PE transpose (bf16), Wo projection (+bias), host
    transposes the output back.
"""

import numpy as np

import concourse.bass as bass
import concourse.bacc as bacc
import concourse.mybir as mybir
from concourse.tile import TileContext
from concourse.bass import ts
from concourse.bass_utils import run_bass_kernel_spmd

F32 = mybir.dt.float32
F32R = mybir.dt.float32r
BF16 = mybir.dt.bfloat16
F8 = mybir.dt.float8e4
DR = mybir.MatmulPerfMode.DoubleRow

B, S, D = 2, 2048, 1024
H, HD = 16, 64
HALF_W = 256          # window // 2: query i attends keys [i-256, i]
TC = 512              # tokens per core
TH = TC + HALF_W      # tokens incl halo = 768
NQB = TC // 128       # query blocks per core = 4
DC = D // 128         # 8 partition chunks of the model dim
NTC = TH // 128       # token chunks incl halo = 6
VW = HD + 1           # per-head V width incl ones column = 65
THETA = 10000.0

QK_FP8 = False        # Q/K projections in fp8 DoubleRow
PP_BUFS, UW_BUFS, CX_BUFS, OP_BUFS = 3, 3, 2, 5
VO_FP8 = False        # V/Wo projections in fp8 DoubleRow
W_SCALE = 32.0        # fp8 weight prescale (folded back out downstream)


def round_fp32r(x: np.ndarray) -> np.ndarray:
    b = np.ascontiguousarray(x, dtype=np.float32).view(np.uint32)
    out = (b + np.uint32(0x7FF) + ((b >> np.uint32(12)) & np.uint32(1))) & np.uint32(
        0xFFFFF000
    )
    return out.view(np.float32)


def build_nc(loop_repeat=None, phases=4):
    nc = bacc.Bacc(None, target_bir_lowering=False)

    exp_scale = 0.125 / (W_SCALE * W_SCALE) if QK_FP8 else 0.125
    out_scale = 1.0 / (W_SCALE * W_SCALE) if VO_FP8 else 1.0

    # ---- dram tensors ----
    if QK_FP8 or VO_FP8:
        xt8 = nc.dram_tensor("xt8", [128, 4, 2, TH], F8, kind="ExternalInput")
    if not (QK_FP8 and VO_FP8):
        xt = nc.dram_tensor("xt", [128, DC, TH], BF16, kind="ExternalInput")
    if QK_FP8:
        wq8 = nc.dram_tensor("wq8", [128, 4 * 2 * D], F8, kind="ExternalInput")
        wk8 = nc.dram_tensor("wk8", [128, 4 * 2 * D], F8, kind="ExternalInput")
    else:
        wq = nc.dram_tensor("wq", [128, DC * D], BF16, kind="ExternalInput")
        wk = nc.dram_tensor("wk", [128, DC * D], BF16, kind="ExternalInput")
    if VO_FP8:
        wv8 = nc.dram_tensor("wv8", [128, 4 * 2 * D], F8, kind="ExternalInput")
        wo8 = nc.dram_tensor("wo8", [128, 4 * 2 * D], F8, kind="ExternalInput")
    else:
        wv = nc.dram_tensor("wv", [128, DC * D], BF16, kind="ExternalInput")
        wo = nc.dram_tensor("wo", [128, DC * D], BF16, kind="ExternalInput")
    tblpk = nc.dram_tensor("tblpk", [128, 4096], BF16, kind="ExternalInput")
    tblsm = nc.dram_tensor("tblsm", [128, 12], F32, kind="ExternalInput")
    tblbf = nc.dram_tensor("tblbf", [128, 1280], BF16, kind="ExternalInput")
    tblon = nc.dram_tensor("tblon", [1, 256], F32R, kind="ExternalInput")
    tblv = nc.dram_tensor("tblv", [128, 96], BF16, kind="ExternalInput")
    outT = nc.dram_tensor("outT", [D, TC], F32, kind="ExternalOutput")

    with TileContext(nc) as tc:
        with (
            tc.tile_pool(name="tbl", bufs=1) as tbl,
            tc.tile_pool(name="qkp", bufs=2) as qkp,
            tc.tile_pool(name="vp", bufs=2) as vp,
            tc.tile_pool(name="xtp", bufs=2) as xtp,
            tc.tile_pool(name="wpool", bufs=4) as wpool,
            tc.tile_pool(name="uwp", bufs=UW_BUFS) as uwp,
            tc.tile_pool(name="pp", bufs=PP_BUFS) as pp,
            tc.tile_pool(name="cxtp", bufs=2) as cxtp,
            tc.tile_pool(name="sm", bufs=6) as sm,
            tc.tile_pool(name="op", bufs=OP_BUFS) as op,
            tc.tile_pool(name="projp", bufs=2, space="PSUM") as projp,
            tc.tile_pool(name="scp", bufs=2, space="PSUM") as scp,
        ):
            # ---- constant/table loads (packed: 3 DMAs) ----
            tblpk_sb = tbl.tile([128, 4096], BF16)
            nc.sync.dma_start(out=tblpk_sb, in_=tblpk[:, :])
            tblsm_sb = tbl.tile([128, 12], F32)
            nc.sync.dma_start(out=tblsm_sb, in_=tblsm[:, :])
            tblbf_sb = tbl.tile([128, 1280], BF16)
            nc.sync.dma_start(out=tblbf_sb, in_=tblbf[:, :])
            tblon_sb = tbl.tile([1, 256], F32R)
            nc.sync.dma_start(out=tblon_sb, in_=tblon[:, :])
            cosq2_sb = tblpk_sb[:, 0:1024]
            sinq2_sb = tblpk_sb[:, 1024:2048]
            cosk2_sb = tblpk_sb[:, 2048:3072].rearrange("p (a b) -> p a b", b=512)
            sink2_sb = tblpk_sb[:, 3072:4096].rearrange("p (a b) -> p a b", b=512)
            corr_sb = tblsm_sb[:, 0:NQB]
            bo_sb = tblsm_sb[:, 4:12]
            ident = tblbf_sb[:, 0:128]
            perm32 = tblbf_sb[:, 128:256]
            m_kc0 = tblbf_sb[:, 256:512]   # -30000 where k < q (window edge)
            m_kc2 = tblbf_sb[:, 512:768]   # -30000 where k > q (causal edge)
            mm2 = tblbf_sb[:, 768:1024].rearrange(
                "p (a b) -> p a b", b=128
            )  # 0/1 mult mask, keep k <= q, dup for 2 heads

            def body():
                # ---- input loads: query cols first so Q-proj starts early
                if QK_FP8 or VO_FP8:
                    xt8_sb = xtp.tile([128, 4, 2, TH], F8, name="xt8_sb")
                    nc.sync.dma_start(
                        out=xt8_sb[:, :, :, HALF_W:TH],
                        in_=xt8[:, :, :, HALF_W:TH],
                    )
                if not (QK_FP8 and VO_FP8):
                    xt_sb = xtp.tile([128, DC, TH], BF16, name="xt_sb")
                    nc.sync.dma_start(
                        out=xt_sb[:, 0, HALF_W:TH], in_=xt[:, 0, HALF_W:TH]
                    )

                def load_w8(w_dram, nm):
                    w_sb = wpool.tile(
                        [128, 4, 2, DC, 128], F8, tag="w8", name=f"w8_{nm}"
                    )
                    nc.sync.dma_start(
                        out=w_sb.rearrange("p a b c d -> p (a b c d)"),
                        in_=w_dram[:, :],
                    )
                    return w_sb

                def load_w_blocked(w_dram, nm, coarse=False):
                    halves = []
                    for hh in range(2):
                        w_sb = wpool.tile(
                            [128, DC // 2, DC, 128], BF16, tag="w", name=f"w_{nm}{hh}"
                        )
                        if coarse:
                            nc.sync.dma_start(
                                out=w_sb.rearrange("p a b c -> p (a b c)"),
                                in_=w_dram[:, hh * 4 * D : (hh * 4 + 4) * D],
                            )
                        else:
                            for dcl in range(DC // 2):
                                off = (hh * 4 + dcl) * D
                                nc.sync.dma_start(
                                    out=w_sb[:, dcl], in_=w_dram[:, off : off + D]
                                )
                        halves.append(w_sb)
                    return lambda k, dc: halves[dc // 4][:, dc % 4, k]

                def load_w_kmajor(w_dram, nm):
                    halves = []
                    for hh in range(2):
                        w_sb = wpool.tile(
                            [128, DC // 2, D], BF16, tag="w", name=f"w_{nm}{hh}"
                        )
                        nc.sync.dma_start(
                            out=w_sb.rearrange("p a b -> p (a b)"),
                            in_=w_dram[:, hh * 4 * D : (hh * 4 + 4) * D],
                        )
                        halves.append(w_sb)
                    return lambda k: halves[k // 4][:, k % 4]

                if QK_FP8:
                    wq_sb = load_w8(wq8, "q")
                else:
                    wq_halves = [
                        wpool.tile(
                            [128, DC // 2, DC, 128], BF16, tag="w", name=f"w_q{hh}"
                        )
                        for hh in range(2)
                    ]
                    nc.sync.dma_start(
                        out=wq_halves[0][:, 0], in_=wq[:, 0:D]
                    )
                    for k in range(1, DC):
                        nc.sync.dma_start(
                            out=xt_sb[:, k, HALF_W:TH], in_=xt[:, k, HALF_W:TH]
                        )
                    for hh in range(2):
                        for dcl in range(DC // 2):
                            if hh == 0 and dcl == 0:
                                continue
                            off = (hh * 4 + dcl) * D
                            nc.sync.dma_start(
                                out=wq_halves[hh][:, dcl],
                                in_=wq[:, off : off + D],
                            )
                    wq_at = lambda k, dc: wq_halves[dc // 4][:, dc % 4, k]

                # remaining halo columns of x
                if QK_FP8 or VO_FP8:
                    nc.sync.dma_start(
                        out=xt8_sb[:, :, :, 0:HALF_W], in_=xt8[:, :, :, 0:HALF_W]
                    )
                if not (QK_FP8 and VO_FP8):
                    for kh in range(2):
                        nc.sync.dma_start(
                            out=xt_sb[:, 4 * kh : 4 * kh + 4, 0:HALF_W],
                            in_=xt[:, 4 * kh : 4 * kh + 4, 0:HALF_W],
                        )

                if QK_FP8:
                    wk_sb = load_w8(wk8, "k")
                else:
                    wk_at = load_w_blocked(wk, "k")

                qrope = qkp.tile([128, DC, TC], BF16, name="qrope")
                krope = qkp.tile([128, DC, TH], BF16, name="krope")

                def rope_fused(ps, cos_ap, sin_ap, out_ap, n):
                    """ps: psum [128, 2, 512] (cols :n used per half).
                    out = ps*cos + shift32(ps*sin2); all views [128, 2, n]."""
                    u = uwp.tile([128, 2, 512], BF16, tag="u")
                    w = uwp.tile([128, 2, 512], BF16, tag="w")
                    pv = ps[:, :, 0:n]
                    nc.vector.scalar_tensor_tensor(
                        out=u[:, :, 0:n], in0=pv, scalar=1.0, in1=cos_ap,
                        op0=mybir.AluOpType.bypass, op1=mybir.AluOpType.mult,
                    )
                    nc.vector.scalar_tensor_tensor(
                        out=w[:, :, 0:n], in0=pv, scalar=1.0, in1=sin_ap,
                        op0=mybir.AluOpType.bypass, op1=mybir.AluOpType.mult,
                    )
                    ws = scp.tile([128, 2, 512], F32, tag="sc")
                    for i in range(2):
                        nc.tensor.matmul(
                            ws[:, i, 0:n], perm32, w[:, i, 0:n],
                            start=True, stop=True,
                        )
                    nc.vector.tensor_add(out_ap, ws[:, :, 0:n], u[:, :, 0:n])

                if VO_FP8:
                    wv_sb = load_w8(wv8, "v")
                else:
                    wv_at = load_w_kmajor(wv, "v")

                # ---- V projection (token-major, 65-wide per-head groups) ----
                # tcn 0..2 emitted here; tcn 3..5 interleave into attention qb0
                # col 64 = per-token validity (0 for zero-padded halo tokens,
                # else 1): the ctx^T matmul's denominator row then counts only
                # real keys, so no post-hoc correction is needed.
                v_sb = vp.tile([128, NTC, H, VW], BF16, name="v_sb")
                nc.sync.dma_start(
                    out=v_sb[:, :, :, HD:VW],
                    in_=tblv[:, :].rearrange("p (a b) -> p a b", b=H),
                )

                def emit_v(tcn):
                    ps = projp.tile([128, 2, 512], F32, tag="proj")
                    for half in range(2):
                        for k in range(DC):
                            nc.tensor.matmul(
                                ps[:, half],
                                xt_sb[:, k, ts(tcn, 128)],
                                wv_at(k)[:, ts(half, 512)],
                                start=(k == 0), stop=(k == DC - 1),
                            )
                        nc.vector.tensor_copy(
                            v_sb[:, tcn, half * 8 : half * 8 + 8, 0:HD],
                            ps[:, half].rearrange("p (h c) -> p h c", c=HD),
                        )

                # ---- Q/K projections + RoPE, software-pipelined: the perm
                # matmul + add of chunk i-1 are emitted after chunk i's
                # projection so the in-order PE never waits on the DVE.
                def rope_mults(ps, cos_ap, sin_ap, n):
                    s_sb = uwp.tile([128, 2, 512], BF16, tag="s")
                    u = uwp.tile([128, 2, 512], BF16, tag="u")
                    w = uwp.tile([128, 2, 512], BF16, tag="w")
                    nc.scalar.copy(out=s_sb[:, :, 0:n], in_=ps[:, :, 0:n])
                    nc.vector.tensor_mul(u[:, :, 0:n], s_sb[:, :, 0:n], cos_ap)
                    nc.vector.tensor_mul(w[:, :, 0:n], s_sb[:, :, 0:n], sin_ap)
                    return u, w

                def rope_combine(u, w, out_ap, n):
                    ws = scp.tile([128, 2, 512], F32, tag="sc")
                    for i in range(2):
                        nc.tensor.matmul(
                            ws[:, i, 0:n], perm32, w[:, i, 0:n],
                            start=True, stop=True,
                        )
                    nc.vector.tensor_add(out_ap, ws[:, :, 0:n], u[:, :, 0:n])

                cosq3 = cosq2_sb.rearrange("p (a b) -> p a b", b=512)
                sinq3 = sinq2_sb.rearrange("p (a b) -> p a b", b=512)

                def emit_qk_proj(i):
                    """i in 0..3: Q dc-pairs; 4..11: K dc chunks (if phase>=2)."""
                    ps = projp.tile([128, 2, 512], F32, tag="proj")
                    if i < 4:
                        for half in range(2):
                            dc = 2 * i + half
                            for k in range(DC):
                                nc.tensor.matmul(
                                    ps[:, half], wq_at(k, dc),
                                    xt_sb[:, k, HALF_W:TH],
                                    start=(k == 0), stop=(k == DC - 1),
                                )
                        u, w = rope_mults(ps, cosq3, sinq3, 512)
                        return (u, w, qrope[:, 2 * i : 2 * i + 2, :], 512)
                    dc = i - 4
                    for half in range(2):
                        cs = slice(half * 384, half * 384 + 384)
                        for k in range(DC):
                            nc.tensor.matmul(
                                ps[:, half, 0:384], wk_at(k, dc),
                                xt_sb[:, k, cs],
                                start=(k == 0), stop=(k == DC - 1),
                            )
                    u, w = rope_mults(
                        ps, cosk2_sb[:, :, 0:384], sink2_sb[:, :, 0:384], 384
                    )
                    return (
                        u, w,
                        krope[:, dc, :].rearrange("p (a b) -> p a b", b=384),
                        384,
                    )

                n_chunks = 12 if phases >= 2 else 4
                pend_rope = None
                v_early = {}
                for i in range(n_chunks):
                    cur = emit_qk_proj(i)
                    if pend_rope is not None:
                        rope_combine(*pend_rope)
                    pend_rope = cur
                    if i in v_early:
                        emit_v(v_early[i])
                if pend_rope is not None:
                    rope_combine(*pend_rope)

                if phases == 2:
                    for tcn in range(NTC):
                        emit_v(tcn)

                if VO_FP8:
                    wo_sb = load_w8(wo8, "o")
                else:
                    wo_at = load_w_blocked(wo, "o", coarse=True)

                if VO_FP8:
                    ctxT8 = cxtp.tile([128, 4, 2, TC], F8, name="ctxT8")
                else:
                    ctxT = cxtp.tile([128, DC, TC], BF16, name="ctxT")

                # ---- attention: software-pipelined over (qb, h2) ----
                # ctx is computed TRANSPOSED (head-dims on partitions): per
                # head, lhsT = v65 (64 data cols + validity col) and rhs = pT,
                # giving psum [65, 128q] per head in col-blocks of one tile;
                # row 64 is the softmax denominator. Two K=1 ones-matmuls
                # broadcast the raw denominators across all 128 partitions,
                # reciprocal_approx_fast (full-lane) fuses recip with the
                # psum->sbuf move, and two DVE mults write normalized ctxT
                # directly -- no PE transposes.

                def emit_scores(qb, h2):
                    psS = scp.tile(
                        [128, 2, 4, 128], F32, tag="sc", name=f"psS_{qb}_{h2}"
                    )
                    for kc in range(3):
                        for hp_i in range(2):
                            hp = 64 * hp_i
                            nc.tensor.matmul(
                                psS[:, hp_i, kc],
                                krope[hp : hp + 64, h2, (qb + kc) * 128 : (qb + kc + 1) * 128],
                                qrope[hp : hp + 64, h2, ts(qb, 128)],
                                start=True, stop=True,
                            )
                    pT = pp.tile(
                        [128, 2, 3, 128], BF16, tag="pT", name=f"pT_{qb}_{h2}"
                    )
                    nc.scalar.activation(
                        pT, psS[:, :, 0:3, :],
                        mybir.ActivationFunctionType.Exp, scale=exp_scale,
                    )
                    nc.gpsimd.affine_select(
                        out=pT[:, :, 0, :], in_=pT[:, :, 0, :],
                        compare_op=mybir.AluOpType.is_ge, fill=0.0,
                        base=0, channel_multiplier=1, pattern=[[0, 2], [-1, 128]],
                    )
                    nc.gpsimd.affine_select(
                        out=pT[:, :, 2, :], in_=pT[:, :, 2, :],
                        compare_op=mybir.AluOpType.is_ge, fill=0.0,
                        base=0, channel_multiplier=-1, pattern=[[0, 2], [1, 128]],
                    )
                    return pT

                def emit_ctx_mm(qb, h2, pT):
                    ct = projp.tile(
                        [128, 3, 128], F32, tag="proj", name=f"ct_{qb}_{h2}"
                    )
                    for hp_i in range(2):
                        h = 2 * h2 + hp_i
                        for kc in range(3):
                            nc.tensor.matmul(
                                ct[0:65, hp_i, :],
                                v_sb[:, qb + kc, h, :],
                                pT[:, hp_i, kc, :],
                                start=(kc == 0), stop=(kc == 2),
                                skip_group_check=True,
                            )
                    den = sm.tile([1, 2, 128], F32R, tag="den")
                    nc.scalar.copy(out=den[0:1, :, :], in_=ct[64:65, 0:2, :])
                    return ct, den

                def emit_ctx_fin(qb, h2, ct, den):
                    for hp_i in range(2):
                        nc.tensor.matmul(
                            ct[:, 2, :],
                            tblon_sb[:, 128 * hp_i : 128 * hp_i + 128],
                            den[0:1, hp_i, :],
                            start=(hp_i == 0), stop=(hp_i == 1),
                            skip_group_check=True,
                        )
                    rb = sm.tile([128, 128], F32, tag="rb")
                    nc.vector.reciprocal_approx_fast(out=rb, in_=ct[:, 2, :])
                    qs = ts(qb, 128)
                    nc.vector.tensor_mul(
                        ctxT[0:64, h2, qs], ct[0:64, 0, :], rb[0:64]
                    )
                    nc.vector.tensor_mul(
                        ctxT[64:128, h2, qs], ct[0:64, 1, :], rb[64:128]
                    )

                def emit_wo(qpo, dco):
                    cs = slice(qpo * 256, qpo * 256 + 256)
                    psO = projp.tile([128, 256], F32, tag="proj")
                    if VO_FP8:
                        for g in range(4):
                            nc.tensor.matmul(
                                psO,
                                wo_sb[:, g, :, dco, :],
                                ctxT8[:, g, :, cs],
                                start=(g == 0), stop=(g == 3),
                                perf_mode=DR,
                            )
                    else:
                        for k in range(DC):
                            nc.tensor.matmul(
                                psO, wo_at(k, dco), ctxT[:, k, cs],
                                start=(k == 0), stop=(k == DC - 1),
                            )
                    o_sb = op.tile([128, 256], F32, tag="o")
                    nc.scalar.activation(
                        o_sb, psO, mybir.ActivationFunctionType.Identity,
                        bias=bo_sb[:, dco : dco + 1], scale=out_scale,
                    )
                    nc.sync.dma_start(out=outT[ts(dco, 128), cs], in_=o_sb)

                from collections import deque
                pend = deque()   # (qb, h2, pT) awaiting ctx matmuls
                fin = deque()    # (qb, h2, ct, den) awaiting normalize
                CTX_LAG = 2
                FIN_LAG = 1
                if phases >= 3:
                    # prefill: first two scores interleave the V projections
                    for j in range(3):
                        emit_v(j)
                        if j < 2:
                            pend.append((0, j, emit_scores(0, j)))
                for qb in range(NQB if phases >= 3 else 0):
                    for h2 in range(DC):
                        if qb == 0 and h2 < 2:
                            continue
                        pT = emit_scores(qb, h2)
                        if len(pend) >= CTX_LAG:
                            a, b, p = pend.popleft()
                            fin.append((a, b) + emit_ctx_mm(a, b, p))
                        if len(fin) > FIN_LAG:
                            emit_ctx_fin(*fin.popleft())
                        pend.append((qb, h2, pT))
                        if qb == 0 and 2 <= h2 < 5:
                            emit_v(1 + h2)   # fills the exp/mask latency
                        if qb == 3 and phases >= 4:
                            emit_wo(0, h2)
                while pend:
                    a, b, p = pend.popleft()
                    fin.append((a, b) + emit_ctx_mm(a, b, p))
                    if len(fin) > FIN_LAG:
                        emit_ctx_fin(*fin.popleft())
                while fin:
                    emit_ctx_fin(*fin.popleft())

                # ---- Wo second half (first half interleaved above) ----
                for qpo in range(1, 2) if phases >= 4 else []:
                    for dco in range(DC):
                        emit_wo(qpo, dco)

            if loop_repeat is None:
                body()
            elif loop_repeat == 0:
                pass  # empty body: measures fixed dispatch/launch overhead
            elif isinstance(loop_repeat, tuple):
                # (u, n): For_i loop of n iterations, each with u unrolled
                # bodies. Comparing wall time at two u values (same n)
                # cancels the per-iteration barrier and host overhead:
                # slope vs u = pipelined per-body steady-state time.
                u, n = loop_repeat
                if u == 0:
                    with tc.For_i(0, n, 1):
                        pass
                else:
                    with tc.For_i(0, n, 1):
                        for _ in range(u):
                            body()
            else:
                with tc.For_i(0, loop_repeat, 1):
                    body()

    nc.compile()
    return nc


_NC_CACHE = None


def _get_nc():
    global _NC_CACHE
    if _NC_CACHE is None:
        _NC_CACHE = build_nc()
    return _NC_CACHE


def _host_tables():
    """RoPE cos/sin tables, dim-major, tiled to 128 partitions (2 heads)."""
    inv_freq = 1.0 / (THETA ** (np.arange(0, HD, 2, dtype=np.float32) / HD))  # [32]
    ifq64 = np.concatenate([inv_freq, inv_freq])  # dim d uses inv_freq[d % 32]

    def tables(positions):
        ang = ifq64[:, None] * positions[None, :].astype(np.float32)  # [64, n]
        cos = np.cos(ang).astype(np.float32)
        sin = np.sin(ang).astype(np.float32)
        sin2 = np.concatenate([sin[:32], -sin[32:]], axis=0)  # sign flip 2nd half
        return np.tile(cos, (2, 1)), np.tile(sin2, (2, 1))

    return tables


def _dc_block(w):
    """[D, D] -> [128, DC*D] with per-partition layout [dc, k, c]."""
    return np.ascontiguousarray(
        np.asarray(w, dtype=np.float32)
        .reshape(DC, 128, DC, 128)
        .transpose(1, 2, 0, 3)
        .reshape(128, DC * D)
    )


def _to_fp8(x):
    return np.asarray(x, dtype=np.float32).astype(mybir.dt.np(F8))


def _pack_dr_w(w):
    """[D, Dout] -> [128ki, 4g*2ko*Dout] fp8, x W_SCALE (d = (2g+ko)*128+ki)."""
    a = np.asarray(w, dtype=np.float32) * W_SCALE
    a = a.reshape(4, 2, 128, -1).transpose(2, 0, 1, 3)  # [128, 4, 2, Dout]
    return _to_fp8(np.ascontiguousarray(a).reshape(128, -1))


def prep_in_maps(input_sequence, Wq, Wk, Wv, Wo, bo):
    x = np.asarray(input_sequence, dtype=np.float32)
    BF = mybir.dt.np(BF16)
    if QK_FP8:
        wq_r = _pack_dr_w(Wq)
        wk_r = _pack_dr_w(Wk)
    else:
        wq_r = _dc_block(Wq).astype(BF)
        wk_r = _dc_block(Wk).astype(BF)
    if VO_FP8:
        wv_r = _pack_dr_w(Wv)
        wo_r = _pack_dr_w(Wo)
    else:
        wv_r = np.ascontiguousarray(
            np.asarray(Wv, dtype=np.float32)
            .reshape(DC, 128, D)
            .transpose(1, 0, 2)
            .reshape(128, DC * D)
        ).astype(BF)
        wo_r = _dc_block(Wo).astype(BF)
    bo_t = np.asarray(bo, dtype=np.float32).reshape(DC, 128).T.copy()

    tables = _host_tables()
    in_maps = []
    for c in range(8):
        b, t = c // 4, c % 4
        start = t * TC
        lo = start - HALF_W
        xt = np.zeros((D, TH), dtype=np.float32)
        vs = max(0, lo)
        xt[:, vs - lo : TH] = x[b, vs : start + TC, :].T
        xt_pk = np.ascontiguousarray(
            xt.reshape(DC, 128, TH).transpose(1, 0, 2)
        )
        cosq_t, sinq2_t = tables(np.arange(start, start + TC))
        cosk_t, sink2_t = tables(np.arange(lo, start + TC))
        # doubled/chunked table layouts for fused rope ops
        cosq2_t = np.ascontiguousarray(np.tile(cosq_t, (1, 2)))
        sinq2_t2 = np.ascontiguousarray(np.tile(sinq2_t, (1, 2)))

        def chunk_k(tb):
            out = np.zeros((128, 2, 512), dtype=np.float32)
            out[:, 0, 0:384] = tb[:, 0:384]
            out[:, 1, 0:384] = tb[:, 384:768]
            return np.ascontiguousarray(out.reshape(128, 1024))

        qpos = np.arange(start, start + TC)
        corr = np.maximum(0, HALF_W - qpos).astype(np.float32).reshape(NQB, 128).T.copy()
        tblpk = np.concatenate(
            [cosq2_t, sinq2_t2, chunk_k(cosk_t), chunk_k(sink2_t)], axis=1
        ).astype(BF)
        tblsm = np.concatenate([corr, bo_t], axis=1).astype(np.float32)
        kk = np.arange(128)
        m0 = np.where(kk[:, None] < kk[None, :], -30000.0, 0.0).astype(np.float32)
        m2 = np.where(kk[:, None] > kk[None, :], -30000.0, 0.0).astype(np.float32)
        mul2 = (kk[:, None] <= kk[None, :]).astype(np.float32)
        tblbf = np.concatenate(
            [
                np.eye(128, dtype=np.float32),
                np.eye(128, dtype=np.float32)[[p ^ 32 for p in range(128)]],
                m0, m0, m2, m2, mul2, mul2, mul2, mul2,
            ],
            axis=1,
        ).astype(mybir.dt.np(BF16))
        tblon = np.zeros((1, 256), dtype=np.float32)
        tblon[0, 0:64] = 1.0     # sel_lo: head-even denom -> partitions 0..63
        tblon[0, 192:256] = 1.0  # sel_hi: head-odd denom -> partitions 64..127
        # per-token validity for the v ones-column (0 on zero-padded halo)
        valid = (
            (lo + np.arange(NTC)[None, :] * 128 + np.arange(128)[:, None]) >= 0
        ).astype(np.float32)  # [128, NTC]
        tblv = np.repeat(valid[:, :, None], H, axis=2).reshape(128, NTC * H)
        m = {
            "tblpk": np.ascontiguousarray(tblpk),
            "tblsm": np.ascontiguousarray(tblsm),
            "tblbf": np.ascontiguousarray(tblbf),
            "tblon": tblon,
            "tblv": np.ascontiguousarray(tblv).astype(mybir.dt.np(BF16)),
        }
        if QK_FP8 or VO_FP8:
            m["xt8"] = _to_fp8(
                np.ascontiguousarray(xt.reshape(4, 2, 128, TH).transpose(2, 0, 1, 3))
            )
        if not (QK_FP8 and VO_FP8):
            m["xt"] = xt_pk.astype(BF)
        if QK_FP8:
            m["wq8"], m["wk8"] = wq_r, wk_r
        else:
            m["wq"], m["wk"] = wq_r, wk_r
        if VO_FP8:
            m["wv8"], m["wo8"] = wv_r, wo_r
        else:
            m["wv"], m["wo"] = wv_r, wo_r
        in_maps.append(m)
    return in_maps


def kernel(input_sequence, Wq, Wk, Wv, Wo, bo):
    nc = _get_nc()
    in_maps = prep_in_maps(input_sequence, Wq, Wk, Wv, Wo, bo)
    res = run_bass_kernel_spmd(nc, in_maps, list(range(8)))
    out = np.empty((B, S, D), dtype=np.float32)
    for c in range(8):
        b, t = c // 4, c % 4
        out[b, t * TC : t * TC + TC, :] = res.results[c]["outT"].T
    return out

